# revision 1
# baseline (speedup 1.0000x reference)
"""Trainium2 Bass kernel for AngularSymmetryMod (ANI-style angular symmetry functions).

Math: out[b,i,l] = sum_{j,k} (1+lam*cos(theta-theta_t))^zeta * exp(-ita*((R_ij+R_ik)/2-Rs)^2)
                            * f_ij*f_ik * 2^(1-zeta)
over a 40-point parameter grid l=(lam in {+-1}, 5 Rs values, 4 theta_t values), zeta=4.

Key optimizations:
 1. theta_t = {0.0, 1.57, 3.14, 4.71} are (to 8e-4) the exact quadrants {0, pi/2, pi, 3pi/2},
    so cos(theta-theta_t) = {c, s, -c, -s} and the angular factor collapses to FOUR distinct
    fields: (1+-c)^4, (1+-s)^4 — each two chained Square activations on the ScalarEngine.
    (Validated: 2.2e-4 rel err vs the f32 reference.)
 2. Each of the 40 outputs is S[r, m] (5 radials x 4 angulars = 20 reductions); the 40 outputs
    are a column remap handled by the output DMA.
 3. The (j,k) summand is symmetric, so only the 528 pairs j<=k are computed (host gathers the
    packed pair layout; off-diagonal weight 2 is folded into the cutoff product on-chip).
 4. sin/cos need exact-range reduction (theta spans +-2.3e6): theta/2pi - round(theta/2pi) via
    the f32 magic-constant round (x+1.5*2^23)-1.5*2^23 on the DVE, then one double-width Sin
    activation over [sin-args | cos-args] hitting the table on [-pi, pi].

Sharding: data-parallel over batch (16 molecules -> 2 per core on 8 cores). No collectives.
Layout per core: 128 partitions = (b_loc:2, i:32, half:2), free = 264 packed (j,k) pairs
(248 off-diagonal + 16 diagonal per half).
"""

import sys
import numpy as np

sys.path.insert(0, "/opt/trn_rl_repo")

from contextlib import ExitStack

import concourse.bass as bass
import concourse.tile as tile
from concourse import bacc, mybir
from concourse.bass_utils import run_bass_kernel_spmd

B, N, L = 16, 32, 40
NCORES = 8
B_LOC = B // NCORES  # 2
P = 128  # partitions = B_LOC * N * 2
NT = 264           # packed pairs per partition-half
NOFF = 248         # off-diagonal entries (first NOFF of NT); rest are diagonal

BOHR = 0.52917721092
ITA = 1.12
ZETA = 4.0
RS_VALS = np.array([0.5, 1.17, 1.83, 2.5, 3.17]) / BOHR
NR, NM = 5, 4

F32 = mybir.dt.float32
I32 = mybir.dt.int32
OP = mybir.AluOpType
ACT = mybir.ActivationFunctionType

# free-axis offsets: [ci(3) | per-coordinate (cj_c, ck_c) pair blocks | u/f block]
OFF_CI = 0                  # [3]    coords of atom i (per-partition scalars)
OFF_C0 = 3                  # [2*NT] (cj_x, ck_x) — then y, z blocks of the same shape
OFF_UJ = 3 + 6 * NT         # [NT]   d[b,i,j_t]
OFF_UK = OFF_UJ + NT        # [NT]   d[b,i,k_t]
OFF_FJ = OFF_UK + NT        # [NT]   d_cutoff[b,i,j_t]
OFF_FK = OFF_FJ + NT        # [NT]   d_cutoff[b,i,k_t]
NIN = 3 + 10 * NT


def _pair_index():
    """Static (j,k) pair enumeration: per half, 248 off-diagonal + 16 diagonal."""
    pairs = [(j, k) for j in range(N) for k in range(j + 1, N)]  # 496
    halves = [pairs[0::2], pairs[1::2]]
    tri_j = np.zeros((2, NT), dtype=np.int64)
    tri_k = np.zeros((2, NT), dtype=np.int64)
    for h in range(2):
        for t, (j, k) in enumerate(halves[h]):
            tri_j[h, t], tri_k[h, t] = j, k
        for t2, j in enumerate(range(h * 16, (h + 1) * 16)):
            tri_j[h, NOFF + t2] = tri_k[h, NOFF + t2] = j
    return tri_j, tri_k


_TRI_J, _TRI_K = _pair_index()


def _build():
    nc = bacc.Bacc("TRN2", target_bir_lowering=False, debug=False)
    inp_d = nc.declare_dram_parameter("inp", [P, NIN], F32, isOutput=False)
    cst_d = nc.declare_dram_parameter("cst", [P, 64], F32, isOutput=False)
    out_d = nc.declare_dram_parameter("out", [B_LOC * N, L], F32, isOutput=True)

    TWO_PI = float(2.0 * np.pi)

    with tile.TileContext(nc) as tc, ExitStack() as ctx:
        pool = ctx.enter_context(tc.tile_pool(name="sb", bufs=1))
        rad_pool = ctx.enter_context(tc.tile_pool(name="rad", bufs=2))
        w_pool = ctx.enter_context(tc.tile_pool(name="w", bufs=2))
        scr_pool = ctx.enter_context(tc.tile_pool(name="scr", bufs=3))
        psum = ctx.enter_context(tc.tile_pool(name="ps", bufs=1, space="PSUM"))

        def big(tag, dt=F32):
            return pool.tile([P, NT], dt, name=tag, tag=tag)

        # chunked input DMAs: each coordinate's (cj_c, ck_c) pair block lands separately
        # so the dot-product chain pipelines behind the transfers.
        cic = pool.tile([P, 3], F32, name="cic", tag="cic")
        geo = [pool.tile([P, 2 * NT], F32, name=f"geo{c}", tag=f"geo{c}") for c in range(3)]
        uf = pool.tile([P, 4 * NT], F32, name="uf", tag="uf")
        cst = pool.tile([P, 64], F32, name="cst", tag="cst")
        nc.sync.dma_start(cic[:], inp_d[:, OFF_CI : OFF_CI + 3])
        for c in range(3):
            nc.sync.dma_start(geo[c][:], inp_d[:, OFF_C0 + 2 * NT * c : OFF_C0 + 2 * NT * (c + 1)])
        nc.gpsimd.dma_start(uf[:], inp_d[:, OFF_UJ : OFF_UJ + 4 * NT])
        nc.gpsimd.dma_start(cst[:], cst_d[:])
        uj = uf[:, 0 * NT : 1 * NT]
        uk = uf[:, 1 * NT : 2 * NT]
        fj = uf[:, 2 * NT : 3 * NT]
        fk = uf[:, 3 * NT : 4 * NT]

        # ---- vj = xj - xi (= -v_j), vk = xk - xi (= -v_k); dot = sum_c vj_c*vk_c ----
        vj = pool.tile([P, 3, NT], F32, name="vj", tag="vj")
        vk = pool.tile([P, 3, NT], F32, name="vk", tag="vk")
        prod = big("prod")
        dot = big("dot")
        for c in range(3):
            nc.vector.tensor_scalar(vj[:, c, :], geo[c][:, 0:NT], cic[:, c : c + 1], None, OP.subtract)
            nc.vector.tensor_scalar(vk[:, c, :], geo[c][:, NT : 2 * NT], cic[:, c : c + 1], None, OP.subtract)
            if c == 0:
                nc.vector.tensor_tensor(dot[:], vj[:, 0, :], vk[:, 0, :], OP.mult)
            else:
                nc.vector.tensor_tensor(prod[:], vj[:, c, :], vk[:, c, :], OP.mult)
                nc.vector.tensor_tensor(dot[:], dot[:], prod[:], OP.add)

        # ---- thp = theta / 2pi  (theta = dot / (uj*uk + 1e-5)) ----
        den = big("den")
        nc.gpsimd.tensor_tensor(den[:], uj, uk, OP.mult)
        nc.vector.tensor_scalar(den[:], den[:], 1e-5, TWO_PI, OP.add, OP.mult)
        rden = big("rden")
        nc.vector.reciprocal_approx_fast(rden[:], den[:])
        thp = big("thp")
        nc.vector.tensor_tensor(thp[:], dot[:], rden[:], OP.mult)

        # ---- radial stage (emitted before trig so ACT groups Exp with Square:
        #      exp_and_friends loads once, then trig_and_small once) ----
        q = big("q")
        nc.gpsimd.tensor_tensor(q[:], uj, uk, OP.add)
        cut = big("cut")
        nc.vector.scalar_tensor_tensor(
            cut[:, :NOFF], fj[:, :NOFF], 0.25, fk[:, :NOFF], OP.mult, OP.mult)
        nc.vector.scalar_tensor_tensor(
            cut[:, NOFF:], fj[:, NOFF:], 0.125, fk[:, NOFF:], OP.mult, OP.mult)
        Ws = []
        exp_insts = []
        for r in range(NR):
            hv = rad_pool.tile([P, NT], F32, name=f"hv{r}", tag="hv")
            nc.vector.tensor_scalar(hv[:], q[:], 0.5, float(-RS_VALS[r]), OP.mult, OP.add)
            sq = rad_pool.tile([P, NT], F32, name=f"sq{r}", tag="sq")
            nc.vector.tensor_tensor(sq[:], hv[:], hv[:], OP.mult)
            rad = rad_pool.tile([P, NT], F32, name=f"rad{r}", tag="rad")
            exp_insts.append(nc.scalar.activation(rad[:], sq[:], ACT.Exp, scale=float(-ITA)))
            W = w_pool.tile([P, NT], F32, name=f"w{r}", tag=f"w{r}")
            nc.gpsimd.tensor_tensor(W[:], cut[:], rad[:], OP.mult)
            Ws.append(W)

        # ---- range-reduce + sin/cos via the Sin table, both chains fused double-width:
        # tp2 = [thp | thp+0.25]; round via f32 magic constant; one Sin over [128, 2*NT] ----
        RC = float(12582912.0)
        tp2 = pool.tile([P, 2, NT], F32, name="tp2", tag="tp2")
        nc.vector.tensor_copy(tp2[:, 0, :], thp[:])
        nc.vector.tensor_scalar(tp2[:, 1, :], thp[:], 0.25, None, OP.add)
        nf2 = pool.tile([P, 2, NT], F32, name="nf2", tag="nf2")
        nc.vector.tensor_scalar(nf2[:], tp2[:], RC, RC, OP.add, OP.subtract)
        fr2 = pool.tile([P, 2, NT], F32, name="fr2", tag="fr2")
        nc.vector.tensor_tensor(fr2[:], tp2[:], nf2[:], OP.subtract)
        cs = pool.tile([P, 2, NT], F32, name="cs", tag="cs")
        sin1_inst = nc.scalar.activation(cs[:], fr2[:], ACT.Sin, scale=TWO_PI)
        sin2_inst = sin1_inst
        s1 = cs[:, 0, :]
        c1 = cs[:, 1, :]

        # ---- 4 angular fields (1+-c)^4, (1+-s)^4 via two chained Squares on ScalarE ----
        bias_one = pool.tile([P, 1], F32, name="bias_one", tag="bias_one")
        nc.vector.memset(bias_one[:], 1.0)
        angs = []
        for nm, src, sc in (("bp", s1, 1.0), ("bm", s1, -1.0), ("ap", c1, 1.0), ("am", c1, -1.0)):
            g = big("g_" + nm)
            nc.scalar.activation(g[:], src, ACT.Square, bias=bias_one[:], scale=sc)
            a = big("ang_" + nm)
            nc.scalar.activation(a[:], g[:], ACT.Square)
            angs.append(a)


        spart = pool.tile([P, 24], F32, name="spart", tag="spart")

        # ---- fused reduces sum_t W_r * ang_m ; angs order (bp,bm,ap,am) -> m col (1,3,0,2)
        for mi, mcol in ((0, 1), (1, 3), (2, 0), (3, 2)):
            for r in range(NR):
                scr = scr_pool.tile([P, NT], F32, name=f"scr{r}{mcol}", tag="scr")
                nc.vector.scalar_tensor_tensor(
                    scr[:], Ws[r][:], 0.0, angs[mi][:], OP.bypass, OP.mult,
                    accum_out=spart[:, r * NM + mcol : r * NM + mcol + 1])

        # ---- combine: pair-sum over half partitions; assemble all 40 l-columns in PSUM
        #      l = lam*20 + r*4 + t ; lam=+1 -> m=t ; lam=-1 -> m=(t+2)%4 ----
        s2p = psum.tile([64, L], F32, name="s2p", tag="s2p")
        sp3 = spart[:, 0 : NR * NM].rearrange("p (r t) -> p r t", r=NR, t=NM)
        nc.tensor.matmul(s2p[:, 0:20], cst[:, 0:64], spart[:, 0 : NR * NM])
        o3 = s2p[:].rearrange("n (g r t) -> n g r t", g=2, r=NR, t=NM)
        nc.tensor.matmul(o3[:, 1, :, 0:2], cst[:, 0:64], sp3[:, :, 2:4])
        nc.tensor.matmul(o3[:, 1, :, 2:4], cst[:, 0:64], sp3[:, :, 0:2])
        s2s = pool.tile([64, L], F32, name="s2s", tag="s2s")
        nc.vector.tensor_copy(s2s[:], s2p[:])
        nc.sync.dma_start(out_d[:], s2s[:])

    nc.compile()
    return nc


def _ensure_ntff_hook():
    """Register the axon NTFF profiling hook if the image lacks antenv.axon_hooks."""
    import types

    try:
        from antenv.axon_hooks import get_axon_ntff_profile_hook
        if get_axon_ntff_profile_hook() is not None:
            return
        have_mod = True
    except ImportError:
        have_mod = False
    try:
        if "/root/.axon_site" not in sys.path:
            sys.path.insert(0, "/root/.axon_site")
        from trn_agent_boot.trn_boot import _ntff_profile_via_ctypes

        hook = _ntff_profile_via_ctypes("/opt/axon/libaxon_pjrt.so")
        if hook is None:
            return
    except Exception:
        return
    if have_mod:
        from antenv import axon_hooks
        axon_hooks.set_axon_ntff_profile_hook(hook)
    else:
        m = types.ModuleType("antenv.axon_hooks")
        _h = [hook]
        m.get_axon_ntff_profile_hook = lambda: _h[0]
        m.set_axon_ntff_profile_hook = lambda h: _h.__setitem__(0, h)
        import antenv
        antenv.axon_hooks = m
        sys.modules["antenv.axon_hooks"] = m


_NC = None


def _get_nc():
    global _NC
    if _NC is None:
        _NC = _build()
    return _NC


def _host_pack(d_cutoff, d, atom_coordinates):
    """Pure gather/replication of raw inputs into the per-core packed layout."""
    d_cutoff = np.ascontiguousarray(d_cutoff, dtype=np.float32)
    d = np.ascontiguousarray(d, dtype=np.float32)
    coords = np.ascontiguousarray(atom_coordinates, dtype=np.float32)

    p = np.arange(P)
    b_of_p = p // (N * 2)          # [P]
    i_of_p = (p // 2) % N          # [P]
    half = p % 2                   # [P]
    jt = _TRI_J[half]              # [P, NT]
    kt = _TRI_K[half]              # [P, NT]

    in_maps = []
    for c in range(NCORES):
        cd = coords[c * B_LOC : (c + 1) * B_LOC]
        dd = d[c * B_LOC : (c + 1) * B_LOC]
        fc = d_cutoff[c * B_LOC : (c + 1) * B_LOC]
        buf = np.empty((P, NIN), dtype=np.float32)
        buf[:, OFF_CI : OFF_CI + 3] = cd[b_of_p, i_of_p]
        cjv = cd[b_of_p[:, None], jt]   # [P, NT, 3]
        ckv = cd[b_of_p[:, None], kt]   # [P, NT, 3]
        for c in range(3):
            buf[:, OFF_C0 + 2 * NT * c : OFF_C0 + 2 * NT * c + NT] = cjv[:, :, c]
            buf[:, OFF_C0 + 2 * NT * c + NT : OFF_C0 + 2 * NT * (c + 1)] = ckv[:, :, c]
        buf[:, OFF_UJ : OFF_UJ + NT] = dd[b_of_p[:, None], i_of_p[:, None], jt]
        buf[:, OFF_UK : OFF_UK + NT] = dd[b_of_p[:, None], i_of_p[:, None], kt]
        buf[:, OFF_FJ : OFF_FJ + NT] = fc[b_of_p[:, None], i_of_p[:, None], jt]
        buf[:, OFF_FK : OFF_FK + NT] = fc[b_of_p[:, None], i_of_p[:, None], kt]
        in_maps.append({"inp": buf, "cst": _const_blob()})
    return in_maps


_CST = None


def _const_blob():
    global _CST
    if _CST is None:
        cst = np.zeros((P, 64), dtype=np.float32)
        cst[:, 0:64] = np.repeat(np.eye(64, dtype=np.float32), 2, axis=0)
        _CST = cst
    return _CST


def kernel(d_cutoff, d, atom_coordinates, _trace=False):
    if _trace:
        _ensure_ntff_hook()
    nc = _get_nc()
    in_maps = _host_pack(d_cutoff, d, atom_coordinates)
    res = run_bass_kernel_spmd(nc, in_maps, core_ids=list(range(NCORES)), trace=_trace)
    out = np.concatenate(
        [res.results[c]["out"].reshape(B_LOC, N, L) for c in range(NCORES)], axis=0
    ).astype(np.float32)
    if _trace:
        kernel._last_results = res
    return out



# revision 6
# speedup vs baseline: 1.2719x; 1.2719x over previous
"""Trainium2 Bass kernel for AngularSymmetryMod (ANI-style angular symmetry functions).

Math: out[b,i,l] = sum_{j,k} (1+lam*cos(theta-theta_t))^zeta * exp(-ita*((R_ij+R_ik)/2-Rs)^2)
                            * f_ij*f_ik * 2^(1-zeta)
over a 40-point parameter grid l=(lam in {+-1}, 5 Rs values, 4 theta_t values), zeta=4.

Key optimizations over the gathered-pair baseline:
 1. Cyclic pair enumeration: pair (j, k=(j+m) mod 32) for m=0..16, j split 16/16 across
    partition halves. All per-pair operand reads become AFFINE access patterns over compact
    per-partition rows: k-side operands are sliding windows Dk[j'+m] of a rotated 32-col row,
    j-side operands are 0-stride broadcasts of a 16-col row. Input shrinks ~12x (no host
    inflation of the pair layout), killing the DMA wall the baseline had.
    Weights: m=0 (diag) 1x, m=1..15 2x (unordered pair symmetry), m=16 1x (enumerated twice).
 2. theta_t = quadrants -> the angular factor collapses to 4 fields (1+-c)^4, (1+-s)^4.
 3. Radial Gaussians via a recurrence: rad_{r+1} = rad_r * exp(ita*dR_r*q) * c_r, so only
    3 Exp activations (rad_0, and 2 step fields for the 2 distinct Rs spacings) instead of 5
    Square+Exp pairs; the chain itself is cheap DVE scalar_tensor_tensor ops.
 4. One activation-table load per function family: all Exp-family ACT ops emitted before the
    Sin-family ops (Square lives in every table).
 5. bf16 (2x DVE throughput) for everything off the theta-critical path: cutoffs, radial
    W chain, angular fields, and the 20 fused multiply-reduce ops. theta path (coords, d,
    dot, reciprocal, range reduction) stays f32 to match the f32 reference bit-closely.
 6. Work split across Pool/ACT/DVE so the DVE critical path is minimized; final 40-column
    assembly + cross-half pair sum via 3 small PE matmuls of a 0/1 pairing matrix.

Sharding: data-parallel over batch (16 molecules -> 2 per core on 8 cores). No collectives.
Layout per core: 128 partitions = (jhalf:2, b_loc:2, i:32), free = (m:17, j':16) = 272.
"""

import sys
import numpy as np

sys.path.insert(0, "/opt/trn_rl_repo")

from contextlib import ExitStack

import concourse.bass as bass
import concourse.tile as tile
from concourse import bacc, mybir
from concourse.ap import AP
from concourse.bass_utils import run_bass_kernel_spmd

B, N, L = 16, 32, 40
NCORES = 8
B_LOC = B // NCORES  # 2
P = 128              # partitions = 2 halves * B_LOC * N
MC = 17              # m blocks (cyclic shift distances 0..16)
JH = 16              # j' per partition-half
NT = MC * JH         # 272 free elements per partition

BOHR = 0.52917721092
ITA = 1.12
RS = (np.array([0.5, 1.17, 1.83, 2.5, 3.17]) / BOHR).astype(np.float64)
TWO_PI = float(2.0 * np.pi)
RC = float(12582912.0)  # 1.5 * 2^23 f32 round-to-int magic constant

# radial chain constants: rad_{r+1} = rad_r * exp(ITA*dR_r*q) * CCH[r]
DR = RS[1:] - RS[:-1]                      # [0]==[2]==[3], [1] differs
KA = float(ITA * DR[0])
KB = float(ITA * DR[1])
CCH = [float(np.exp(-ITA * DR[r] * (RS[r] + RS[r + 1]))) for r in range(4)]
ECH = [0, 1, 0, 0]  # which step field (Ea/Eb) each chain step uses

# f32 input column offsets
OFF_CI = 0
OFF_DJ = 3
OFF_DK = 19
OFF_CJ = 51   # 3 x 16, coordinate-major
OFF_CK = 99   # 3 x 32
NIN = 195

F32 = mybir.dt.float32
BF16 = mybir.dt.bfloat16
OP = mybir.AluOpType
ACT = mybir.ActivationFunctionType


def _win(t, col_off, m_stride, m_cnt=MC, j_cnt=JH):
    """Affine (m, j') access pattern over a compact per-partition row of tile t.
    m_stride=1 -> sliding window (k-side); m_stride=0 -> broadcast (j-side)."""
    base = t[:]
    part = list(base.ap[0])
    return AP(base.tensor, base.offset + col_off, [part, [m_stride, m_cnt], [1, j_cnt]])


def _build():
    nc = bacc.Bacc("TRN2", target_bir_lowering=False, debug=False)
    inp_d = nc.declare_dram_parameter("inp", [P, NIN], F32, isOutput=False)
    inpb_d = nc.declare_dram_parameter("inpb", [P, 48], BF16, isOutput=False)
    cst_d = nc.declare_dram_parameter("cst", [P, 64], F32, isOutput=False)
    out_d = nc.declare_dram_parameter("out", [B_LOC * N, L], F32, isOutput=True)

    with tile.TileContext(nc) as tc, ExitStack() as ctx:
        pool = ctx.enter_context(tc.tile_pool(name="sb", bufs=1))
        scr_pool = ctx.enter_context(tc.tile_pool(name="scr", bufs=4))
        psum = ctx.enter_context(tc.tile_pool(name="ps", bufs=1, space="PSUM"))

        def big(tag, dt=F32):
            return pool.tile([P, MC, JH], dt, name=tag, tag=tag)

        raw = pool.tile([P, NIN], F32, name="raw", tag="raw")
        rawb = pool.tile([P, 48], BF16, name="rawb", tag="rawb")
        cst = pool.tile([P, 64], F32, name="cst", tag="cst")
        nc.sync.dma_start(raw[:, 0:OFF_CK], inp_d[:, 0:OFF_CK])
        nc.sync.dma_start(raw[:, OFF_CK:NIN], inp_d[:, OFF_CK:NIN])
        nc.sync.dma_start(rawb[:], inpb_d[:])
        nc.sync.dma_start(cst[:], cst_d[:])

        ci = [raw[:, c : c + 1] for c in range(3)]
        Dj_b = _win(raw, OFF_DJ, 0)
        Dk_w = _win(raw, OFF_DK, 1)
        Fj_b = _win(rawb, 0, 0)
        Fk_w = _win(rawb, 16, 1)

        # ---------------- Pool: q, den, cut, W0, tail reduces ----------------
        q3 = big("q3")
        nc.gpsimd.tensor_tensor(q3[:], Dj_b, Dk_w, OP.add)
        den = big("den")
        nc.gpsimd.tensor_tensor(den[:], Dj_b, Dk_w, OP.mult)
        # cut = fj * fk (weights: global 2/8 folded into rad0's exp bias; the m=0/m=16
        # 1/8 blocks get a 0.5x fixup applied to W0 slices on DVE)
        cut = big("cut", BF16)
        nc.gpsimd.tensor_tensor(cut[:], Fj_b, Fk_w, OP.mult)

        # ---------------- ACT: exp-family block (one table) ----------------
        b_mrs0 = pool.tile([P, 1], F32, name="b_mrs0", tag="b_mrs0")
        nc.vector.memset(b_mrs0[:], float(-RS[0]))
        sq0 = big("sq0")
        nc.scalar.activation(sq0[:], q3[:], ACT.Square, bias=b_mrs0[:], scale=0.5)
        b_l4 = pool.tile([P, 1], F32, name="b_l4", tag="b_l4")
        nc.vector.memset(b_l4[:], float(np.log(0.25)))
        rad0 = big("rad0", BF16)
        nc.scalar.activation(rad0[:], sq0[:], ACT.Exp, scale=float(-ITA), bias=b_l4[:])
        Ea = big("Ea", BF16)
        nc.scalar.activation(Ea[:], q3[:], ACT.Exp, scale=KA)
        Eb = big("Eb", BF16)
        nc.scalar.activation(Eb[:], q3[:], ACT.Exp, scale=KB)

        # W0 on pool (ready well before the reduce stage)
        Ws = [big(f"w{r}", BF16) for r in range(5)]
        nc.gpsimd.tensor_tensor(Ws[0][:], cut[:], rad0[:], OP.mult)

        # ---------------- DVE: theta path ----------------
        Vj = pool.tile([P, 3, JH], F32, name="Vj", tag="Vj")
        Vk = pool.tile([P, 3, 32], F32, name="Vk", tag="Vk")
        dparts = []
        for c in range(3):
            nc.vector.tensor_scalar(
                Vk[:, c, :], raw[:, OFF_CK + 32 * c : OFF_CK + 32 * (c + 1)],
                ci[c], None, OP.subtract)
            nc.vector.tensor_scalar(
                Vj[:, c, :], raw[:, OFF_CJ + 16 * c : OFF_CJ + 16 * (c + 1)],
                ci[c], None, OP.subtract)
        da = big("da")
        db = big("db")
        dc = big("dc")
        nc.vector.tensor_tensor(da[:], _win(Vj, 0, 0), _win(Vk, 0, 1), OP.mult)
        nc.vector.tensor_tensor(db[:], _win(Vj, JH, 0), _win(Vk, 32, 1), OP.mult)
        nc.vector.tensor_tensor(dc[:], _win(Vj, 2 * JH, 0), _win(Vk, 64, 1), OP.mult)
        dxy = big("dxy")
        nc.vector.tensor_tensor(dxy[:], da[:], db[:], OP.add)
        dot = big("dot")
        nc.vector.tensor_tensor(dot[:], dxy[:], dc[:], OP.add)

        denp = big("denp")
        nc.vector.tensor_scalar(denp[:], den[:], 1e-5, TWO_PI, OP.add, OP.mult)
        rden = big("rden")
        nc.vector.reciprocal_approx_fast(rden[:], denp[:])
        cs2 = pool.tile([P, 2, MC, JH], F32, name="cs2", tag="cs2")
        nc.vector.tensor_tensor(cs2[:, 0], dot[:], rden[:], OP.mult)
        nc.vector.tensor_scalar(cs2[:, 1], cs2[:, 0], 0.25, None, OP.add)
        nf2 = pool.tile([P, 2, MC, JH], F32, name="nf2", tag="nf2")
        nc.vector.tensor_scalar(nf2[:], cs2[:], RC, RC, OP.add, OP.subtract)
        fr2 = pool.tile([P, 2, MC, JH], F32, name="fr2", tag="fr2")
        nc.vector.tensor_tensor(fr2[:], cs2[:], nf2[:], OP.subtract)

        # ---------------- ACT: trig-family block (second table) ----------------
        sc = pool.tile([P, 2, MC, JH], F32, name="sc", tag="sc")
        nc.scalar.activation(sc[:], fr2[:], ACT.Sin, scale=TWO_PI)
        # 4 angular first-squares (1 +- u)^2 ; Square is in the trig table
        b_one = pool.tile([P, 1], F32, name="b_one", tag="b_one")
        nc.vector.memset(b_one[:], 1.0)
        g2 = [big(f"g2{f}", BF16) for f in range(4)]  # order: cp, sp, cm, sm
        for gi, (half, scl) in enumerate(((1, 1.0), (0, 1.0), (1, -1.0), (0, -1.0))):
            nc.scalar.activation(g2[gi][:], sc[:, half], ACT.Square,
                                 bias=b_one[:], scale=scl)

        # ---------------- DVE: W chain (fills the SIN gap) ----------------
        # m=0 (diag) and m=16 (double-enumerated) blocks carry weight 1/8 not 2/8
        nc.vector.tensor_scalar(Ws[0][:, 0, :], Ws[0][:, 0, :], 0.5, None, OP.mult)
        nc.vector.tensor_scalar(Ws[0][:, 16, :], Ws[0][:, 16, :], 0.5, None, OP.mult)
        for r in range(4):
            E = Eb if ECH[r] else Ea
            nc.vector.scalar_tensor_tensor(
                Ws[r + 1][:], Ws[r][:], CCH[r], E[:], OP.mult, OP.mult)

        # ---------------- fields + 20 fused multiply-reduces ----------------
        spart = pool.tile([P, 24], F32, name="spart", tag="spart")
        av = [big(f"a{f}", BF16) for f in range(4)]
        for f in range(4):
            nc.vector.tensor_tensor(av[f][:], g2[f][:], g2[f][:], OP.mult)
            for r in range(5):
                scr = scr_pool.tile([P, MC, JH], BF16, name=f"scr{r}{f}", tag="scr")
                nc.vector.scalar_tensor_tensor(
                    scr[:], Ws[r][:], 0.0, av[f][:], OP.bypass, OP.mult,
                    accum_out=spart[:, r * 4 + f : r * 4 + f + 1])

        # ---------------- combine: half pair-sum + 40-column assembly ----------------
        s2p = psum.tile([64, L], F32, name="s2p", tag="s2p")
        sp3 = spart[:, 0:20].rearrange("p (r t) -> p r t", r=5, t=4)
        nc.tensor.matmul(s2p[:, 0:20], cst[:, 0:64], spart[:, 0:20])
        o3 = s2p[:].rearrange("n (g r t) -> n g r t", g=2, r=5, t=4)
        nc.tensor.matmul(o3[:, 1, :, 0:2], cst[:, 0:64], sp3[:, :, 2:4])
        nc.tensor.matmul(o3[:, 1, :, 2:4], cst[:, 0:64], sp3[:, :, 0:2])
        s2s = pool.tile([64, L], F32, name="s2s", tag="s2s")
        nc.vector.tensor_copy(s2s[:], s2p[:])
        nc.sync.dma_start(out_d[:], s2s[:])

    nc.compile()
    return nc


def _ensure_ntff_hook():
    """Register the axon NTFF profiling hook if the image lacks antenv.axon_hooks."""
    import types

    try:
        from antenv.axon_hooks import get_axon_ntff_profile_hook
        if get_axon_ntff_profile_hook() is not None:
            return
        have_mod = True
    except ImportError:
        have_mod = False
    try:
        if "/root/.axon_site" not in sys.path:
            sys.path.insert(0, "/root/.axon_site")
        from trn_agent_boot.trn_boot import _ntff_profile_via_ctypes

        hook = _ntff_profile_via_ctypes("/opt/axon/libaxon_pjrt.so")
        if hook is None:
            return
    except Exception:
        return
    if have_mod:
        from antenv import axon_hooks
        axon_hooks.set_axon_ntff_profile_hook(hook)
    else:
        m = types.ModuleType("antenv.axon_hooks")
        _h = [hook]
        m.get_axon_ntff_profile_hook = lambda: _h[0]
        m.set_axon_ntff_profile_hook = lambda h: _h.__setitem__(0, h)
        import antenv
        antenv.axon_hooks = m
        sys.modules["antenv.axon_hooks"] = m


_NC = None


def _get_nc():
    global _NC
    if _NC is None:
        _NC = _build()
    return _NC


# static gather indices (host pack is pure gather/replication of raw inputs)
_pp = np.arange(P)
_H = _pp // 64           # partition half -> j base 16h
_Bp = (_pp // 32) % 2    # local molecule
_Ip = _pp % 32           # atom i
_JBASE = 16 * _H
_JIDX = (_JBASE[:, None] + np.arange(JH)[None, :])            # [P,16] j = 16h+j'
_KIDX = (_JBASE[:, None] + np.arange(32)[None, :]) % 32       # [P,32] rotated k row


def _host_pack(d_cutoff, d, atom_coordinates):
    import ml_dtypes

    d = np.ascontiguousarray(d, dtype=np.float32)
    fc = np.ascontiguousarray(d_cutoff, dtype=np.float32)
    xs = np.ascontiguousarray(atom_coordinates, dtype=np.float32)

    in_maps = []
    for core in range(NCORES):
        dd = d[core * B_LOC : (core + 1) * B_LOC]
        ff = fc[core * B_LOC : (core + 1) * B_LOC]
        xx = xs[core * B_LOC : (core + 1) * B_LOC]
        buf = np.empty((P, NIN), dtype=np.float32)
        buf[:, OFF_CI : OFF_CI + 3] = xx[_Bp, _Ip]
        buf[:, OFF_DJ : OFF_DJ + JH] = dd[_Bp[:, None], _Ip[:, None], _JIDX]
        buf[:, OFF_DK : OFF_DK + 32] = dd[_Bp[:, None], _Ip[:, None], _KIDX]
        for c in range(3):
            buf[:, OFF_CJ + JH * c : OFF_CJ + JH * (c + 1)] = xx[_Bp[:, None], _JIDX, c]
            buf[:, OFF_CK + 32 * c : OFF_CK + 32 * (c + 1)] = xx[_Bp[:, None], _KIDX, c]
        bufb = np.empty((P, 48), dtype=np.float32)
        bufb[:, 0:16] = ff[_Bp[:, None], _Ip[:, None], _JIDX]
        bufb[:, 16:48] = ff[_Bp[:, None], _Ip[:, None], _KIDX]
        in_maps.append({
            "inp": buf,
            "inpb": bufb.astype(ml_dtypes.bfloat16),
            "cst": _const_blob(),
        })
    return in_maps


_CST = None


def _const_blob():
    global _CST
    if _CST is None:
        cst = np.zeros((P, 64), dtype=np.float32)
        cst[np.arange(P), np.arange(P) % 64] = 1.0
        _CST = cst
    return _CST


def kernel(d_cutoff, d, atom_coordinates, _trace=False):
    if _trace:
        _ensure_ntff_hook()
    nc = _get_nc()
    in_maps = _host_pack(d_cutoff, d, atom_coordinates)
    res = run_bass_kernel_spmd(nc, in_maps, core_ids=list(range(NCORES)), trace=_trace)
    out = np.concatenate(
        [res.results[c]["out"].reshape(B_LOC, N, L) for c in range(NCORES)], axis=0
    ).astype(np.float32)
    if _trace:
        kernel._last_results = res
    return out
